# revision 6
# baseline (speedup 1.0000x reference)
"""Multi-head causal attention (B=2, T=2048, E=768, H=12, D=64) on 8 trn2 cores.

Sharding: core c handles batch b=c//4 and heads [3g, 3g+1, 3g+2] (g=c%4).
Each core computes its 3 heads' attention plus their partial contribution to
the final projection; the host sums the 4 partials per batch.

v2 redesign (vs v1):
- all matmuls bf16 (1 cyc/row at any moving size; no fp32r small-N penalty)
- v computed directly in [s, d] layout (no PE transposes); v bias folded into
  phase 3 via a constant ones row in ot2e and a host-precomputed bv@wf row
- causal mask added via PE matmul (ident^T @ mask) inside the S^T psum
  accumulation group instead of a DVE tensor_tensor
- exp instructions batched over pairs of key blocks (2-bank PSUM st tiles)
- DMA issue on sync + gpsimd queues (keeps Act SEQ free for exp)
- phase-3 ob moves split DVE (cols 0:512) / Act (cols 512:768)
- fine-grained emission interleave (attention vs projection/phase-3 filler)
  to keep the PE stream dense

Per-core program:
  phase 1: qT/kT = Wqk^T x^T + b, column groups [q0 q1][q2 k0][k1 k2];
           v[s,d] = x W_v^T per 128-query block (3 heads side by side)
  phase 2: per head h, key-block j: S^T_j = k_j^T q (K=64), +mask on diagonal
           blocks via matmul, P = exp(scale*S^T) (Act, pair-batched),
           [O^T; l] accumulated via matmul(lhsT=[v_j | 1], rhs=P).
           recip = 1/l (DVE), partition-broadcast via K=1 matmul, multiply.
  phase 3: out = [ot01; ot2e]^T @ [wf01; wf2e] -> [2048, 768] partial, DMA.

`repeat` unrolls the whole body N times in one NEFF; test.py measures
per-body HW time as the slope of wall time vs repeat count.
"""
import numpy as np

EMBED_DIM = 768
B = 2
T = 2048
N_CORES = 8
NT = T // 128           # 16 query/key tiles
SCALE = 1.0 / np.sqrt(64.0)
NEG = -1.0e9


_state = {}


def _build(repeat=1):
    import concourse.tile as tile
    from concourse import bacc, mybir
    from concourse.masks import make_identity

    F32 = mybir.dt.float32
    BF16 = mybir.dt.bfloat16

    nc = bacc.Bacc("TRN2", target_bir_lowering=False, debug=False)

    xT_d = nc.dram_tensor("xT", [EMBED_DIM, T], BF16, kind="ExternalInput").ap()
    # columns ordered [q0 q1 | k0 k1 | q2 k2]
    wqk_d = nc.dram_tensor("wqk", [EMBED_DIM, 384], BF16, kind="ExternalInput").ap()
    wv_d = nc.dram_tensor("wv", [EMBED_DIM, 192], BF16, kind="ExternalInput").ap()
    bqk_d = nc.dram_tensor("bqk", [384, 1], F32, kind="ExternalInput").ap()
    wf01_d = nc.dram_tensor("wf01", [128, EMBED_DIM], BF16, kind="ExternalInput").ap()
    wf2e_d = nc.dram_tensor("wf2e", [65, EMBED_DIM], BF16, kind="ExternalInput").ap()
    mask_d = nc.dram_tensor("mask", [128, 128], BF16, kind="ExternalInput").ap()
    out_d = nc.dram_tensor("out_p", [T, EMBED_DIM], BF16, kind="ExternalOutput").ap()

    with tile.TileContext(nc) as tc:
        with tc.tile_pool(name="const", bufs=1) as const, \
             tc.tile_pool(name="persist", bufs=1) as persist:
            # ---- constants ----
            wqk_sb = const.tile([128, 6, 384], BF16)
            wv_sb = const.tile([128, 6, 192], BF16)
            nc.sync.dma_start(out=wqk_sb[:], in_=wqk_d.rearrange("(k p) c -> p k c", p=128))
            nc.gpsimd.dma_start(out=wv_sb[:], in_=wv_d.rearrange("(k p) c -> p k c", p=128))
            bqk_sb = [const.tile([128, 1], F32, name=f"bqk{m}", tag=f"bqk{m}")
                      for m in range(3)]
            for m in range(3):
                nc.sync.dma_start(out=bqk_sb[m][:], in_=bqk_d[128 * m:128 * (m + 1), :])
            wf01_sb = const.tile([128, EMBED_DIM], BF16)
            wf2e_sb = const.tile([65, EMBED_DIM], BF16)
            nc.gpsimd.dma_start(out=wf01_sb[:], in_=wf01_d[:])
            nc.gpsimd.dma_start(out=wf2e_sb[:], in_=wf2e_d[:])
            mask_sb = const.tile([128, 128], BF16)
            nc.sync.dma_start(out=mask_sb[:], in_=mask_d[:])
            ident_f = const.tile([128, 128], F32)
            make_identity(nc, ident_f)
            ident_bf = const.tile([128, 128], BF16)
            nc.vector.tensor_copy(out=ident_bf[:], in_=ident_f[:])
            ones_bf = const.tile([65, 64], BF16)
            nc.vector.memset(ones_bf[:], 1.0)

            # ---- persistent activations ----
            qA = persist.tile([128, T], BF16)    # q0 @0:64, q1 @64:128
            kA = persist.tile([128, T], BF16)    # k0 @0:64, k1 @64:128
            qB = persist.tile([64, T], BF16)     # q2
            kB = persist.tile([64, T], BF16)     # k2
            v_all = persist.tile([128, NT, 3, 65], BF16)   # [v | 1] per head
            nc.vector.memset(v_all[:, :, :, 64:65], 1.0)
            ot01 = persist.tile([128, T], BF16)  # normalized O^T h0 (@0), h1 (@64)
            ot2e = persist.tile([65, T], BF16)   # h2 @0:64; row 64 = ones
            nc.vector.memset(ot2e[64:65, :], 1.0)

            with tc.tile_pool(name="sb", bufs=1) as sbp, \
                 tc.tile_pool(name="ps", bufs=1, space="PSUM") as psp:
                carry = None
                for rep in range(repeat):
                    env = locals()
                    env.update(env.pop("env", {}) or {})
                    carry = _emit_body(nc, tc, rep, env, carry,
                                       last=(rep == repeat - 1))

    nc.compile()
    return nc


def _emit_body(nc, tc, rep, env, carry=None, last=True):
    """Emit one forward pass with fine-grained interleaving.

    `carry` holds the previous body's deferred phase-3 tail items; they are
    emitted after this body's first projection stage so the PE never idles
    waiting for the previous body's final softmax norms.  Returns this
    body's deferred items (or emits them when `last`).

    PSUM tags (8 banks): ps0 ps1 (projection/v/bc groups), stp0 stp1
    (2-bank S^T pair tiles), otl0 otl1 (PV accumulators + phase-3)."""
    from concourse import mybir

    F32 = mybir.dt.float32
    BF16 = mybir.dt.bfloat16
    Exp = mybir.ActivationFunctionType.Exp
    MULT = mybir.AluOpType.mult

    xT_d, out_d = env["xT_d"], env["out_d"]
    wqk_sb, wv_sb = env["wqk_sb"], env["wv_sb"]
    bqk_sb = env["bqk_sb"]
    wf01_sb, wf2e_sb = env["wf01_sb"], env["wf2e_sb"]
    ident_bf, ones_bf, mask_sb = env["ident_bf"], env["ones_bf"], env["mask_sb"]
    qA, kA, qB, kB = env["qA"], env["kA"], env["qB"], env["kB"]
    v_all = env["v_all"]
    ot01, ot2e = env["ot01"], env["ot2e"]
    dmae = [nc.sync, nc.gpsimd]
    sbp, psp = env["sbp"], env["psp"]

    if True:
        # ---- input DMA: xT as 6x4 chunks spread over 2 DMA queues ----
        # (first body issues its own; later bodies use tiles prefetched by
        # the previous body so the data is resident at body start)
        def xT_issue():
            tiles = [[sbp.tile([128, 512], BF16, name=f"xT{rep}_{k}_{n}",
                               tag=f"xT{k}{n}") for n in range(4)]
                     for k in range(6)]
            di = 0
            for n in range(4):
                for k in range(6):
                    dmae[di % 2].dma_start(
                        out=tiles[k][n][:],
                        in_=xT_d[128 * k:128 * (k + 1),
                                 512 * n:512 * (n + 1)])
                    di += 1
            return tiles

        if carry is not None and carry[1] is not None:
            xT_t = carry[1]
        else:
            xT_t = xT_issue()

        gidx = [0]

        def qk_group(m, n):
            ps = psp.tile([128, 512], F32, name=f"pg{rep}_{gidx[0]}",
                          tag=f"ps{gidx[0] % 2}")
            gidx[0] += 1
            for k in range(6):
                nc.tensor.matmul(ps[:], lhsT=wqk_sb[:, k, 128 * m:128 * (m + 1)],
                                 rhs=xT_t[k][n][:], start=(k == 0), stop=(k == 5))
            nsl = slice(512 * n, 512 * (n + 1))
            if m < 2:
                dst = qA if m == 0 else kA
                nc.vector.tensor_scalar_add(out=dst[:, nsl], in0=ps[:],
                                            scalar1=bqk_sb[m][:])
            else:
                nc.vector.tensor_scalar_add(out=qB[:, nsl], in0=ps[0:64, :],
                                            scalar1=bqk_sb[2][0:64, :])
                nc.vector.tensor_scalar_add(out=kB[:, nsl], in0=ps[64:128, :],
                                            scalar1=bqk_sb[2][64:128, :])

        def v_block(i):
            # v[s, d] for s-block i, 3 heads side by side: [128, 192]
            n, off = divmod(128 * i, 512)
            ps = psp.tile([128, 512], F32, name=f"vp{rep}_{i}",
                          tag=f"ps{gidx[0] % 2}")
            gidx[0] += 1
            for k in range(6):
                nc.tensor.matmul(ps[:, 0:192], lhsT=xT_t[k][n][:, off:off + 128],
                                 rhs=wv_sb[:, k, :], start=(k == 0), stop=(k == 5))
            nc.vector.tensor_copy(
                out=v_all[:, i, :, 0:64],
                in_=ps[:, 0:192].rearrange("p (h d) -> p h d", h=3))

        # h -> (qT tile, q part offset, kT tile, k part offset)
        head_cfg = [(qA, 0, kA, 0), (qA, 64, kA, 64), (qB, 0, kB, 0)]
        sidx = [0]
        otli = [0]
        attn_st = {}

        def attn_S(h, q, inject=None):
            """S^T matmuls + diag mask + pair-batched exp for quarter q.
            inject: {pair_index: item} emitted after that pair, to absorb the
            exp pipeline phase lag without displacing the S stream."""
            qT, oq, kT, ok = head_cfg[h]
            base = 512 * q
            pairs = []
            for p in range(2 * q + 2):
                if inject and p in inject:
                    inject.pop(p)()
                st2 = psp.tile([128, 2, 512], F32, name=f"st{rep}_{h}{q}{p}",
                               tag=f"stp{sidx[0] % 2}")
                pt2 = sbp.tile([128, 2, 512], BF16, name=f"pt{rep}_{h}{q}{p}",
                               tag=f"pt{sidx[0] % 6}")
                sidx[0] += 1
                lns = []
                diags = []
                for jj in range(2):
                    j = 2 * p + jj
                    s0 = max(base, 128 * j)
                    ln = base + 512 - s0
                    lns.append(ln)
                    if 128 * j >= base:
                        diags.append(jj)
                    nc.tensor.matmul(
                        st2[:, jj, 0:ln],
                        lhsT=kT[ok:ok + 64, 128 * j:128 * (j + 1)],
                        rhs=qT[oq:oq + 64, s0:s0 + ln],
                        start=True, stop=True)
                mx = max(lns)
                nc.scalar.activation(out=pt2[:, :, 0:mx], in_=st2[:, :, 0:mx],
                                     func=Exp, scale=float(SCALE))
                for jj in diags:
                    nc.vector.tensor_tensor(
                        out=pt2[:, jj, 0:128], in0=pt2[:, jj, 0:128],
                        in1=mask_sb[:], op=MULT)
                pairs.append((st2, pt2, lns))
            attn_st[(h, q)] = pairs

        def attn_PV(h, q):
            base = 512 * q
            pairs = attn_st.pop((h, q))
            otl = psp.tile([128, 512], F32, name=f"otl{rep}_{h}{q}",
                           tag=f"otl{otli[0] % 2}")
            otli[0] += 1
            jmax = 4 * q + 3
            for j in range(jmax + 1):
                s0 = max(base, 128 * j)
                ln = base + 512 - s0
                pt2 = pairs[j // 2][1]
                nc.tensor.matmul(
                    otl[0:65, s0 - base:512],
                    lhsT=v_all[:, j, h, :], rhs=pt2[:, j % 2, 0:ln],
                    start=(j == 0), stop=(j == jmax))
            return otl

        def attn_norm(h, q, otl):
            base = 512 * q
            recip = sbp.tile([65, 512], F32, name=f"rc{rep}_{h}{q}",
                             tag=f"rc{(3 * q + h) % 2}")
            nc.vector.reciprocal(out=recip[0:1, :], in_=otl[64:65, :])
            bcs = sbp.tile([64, 512], F32, name=f"bcs{rep}_{h}{q}",
                           tag=f"bcs{(3 * q + h) % 2}")
            nc.gpsimd.partition_broadcast(bcs[:, :], recip[0:1, :])
            dst, od = (ot2e, 0) if h == 2 else (ot01, 64 * h)
            nc.vector.tensor_tensor(
                out=dst[od:od + 64, base:base + 512],
                in0=otl[0:64, :], in1=bcs[:], op=MULT)

        p3_ob = {}

        def phase3a(i, on_act=False):
            fpa = psp.tile([128, 512], F32, name=f"fpa{rep}_{i}", tag=f"otl{i % 2}")
            ti = slice(128 * i, 128 * (i + 1))
            nc.tensor.matmul(fpa[:], lhsT=ot01[:, ti],
                             rhs=wf01_sb[:, 0:512], start=True, stop=False)
            nc.tensor.matmul(fpa[:], lhsT=ot2e[:, ti],
                             rhs=wf2e_sb[:, 0:512], start=False, stop=True)
            ob = sbp.tile([128, EMBED_DIM], BF16, name=f"ob{rep}_{i}",
                          tag=f"ob{i % 6}")
            p3_ob[i] = ob
            if on_act:
                nc.scalar.copy(out=ob[:, 0:512], in_=fpa[:, :])
            else:
                nc.vector.tensor_copy(out=ob[:, 0:512], in_=fpa[:, :])

        def phase3b(i, on_act=False):
            fpb = psp.tile([128, 512], F32, name=f"fpb{rep}_{i}", tag=f"otl{i % 2}")
            ti = slice(128 * i, 128 * (i + 1))
            nc.tensor.matmul(fpb[:, 0:256], lhsT=ot01[:, ti],
                             rhs=wf01_sb[:, 512:768], start=True, stop=False)
            nc.tensor.matmul(fpb[:, 0:256], lhsT=ot2e[:, ti],
                             rhs=wf2e_sb[:, 512:768], start=False, stop=True)
            ob = p3_ob.pop(i)
            if on_act:
                nc.scalar.copy(out=ob[:, 512:768], in_=fpb[:, 0:256])
            else:
                nc.vector.tensor_copy(out=ob[:, 512:768], in_=fpb[:, 0:256])
            dmae[i % 2].dma_start(out=out_d[ti, :], in_=ob[:])

        # ---- staged emission ----
        # Safety rule: attention quarters are emitted as contiguous blocks
        # (their st/pt/otl tag rotations + cross-engine deps form cycles if
        # other otl-tag users are woven in).  Projections and phase-3 use
        # disjoint tag sets, so they can interleave with each other freely.
        def attn_quarter(q, prelude=(), inject=None):
            """Attention for quarter q, inter-head pipelined.  `prelude` items
            (phase-3 leftovers) are emitted after the first S batch so their
            psum-tag waits hide behind this quarter's exp latency."""
            otl = {}
            attn_S(0, q, inject)
            for it in prelude:
                it()
            attn_S(1, q)
            otl[0] = attn_PV(0, q)
            attn_S(2, q)
            otl[1] = attn_PV(1, q)
            attn_norm(0, q, otl.pop(0))
            otl[2] = attn_PV(2, q)
            attn_norm(1, q, otl.pop(1))
            attn_norm(2, q, otl.pop(2))

        def proj_items(n):
            items = [lambda m=m: qk_group(m, n) for m in range(3)]
            items += [lambda i=i: v_block(i) for i in range(4 * n, 4 * n + 4)]
            return items

        def phase3_items(i0, i1, defer_tail=False, on_act=False):
            """a/b sub-items pipelined: a(i), a(i+1), b(i), a(i+2), b(i+1)...
            With defer_tail, the trailing b-items are split off for the
            caller to emit later."""
            out = []
            pend = []
            for i in range(i0, i1):
                out.append(lambda i=i: phase3a(i, on_act))
                pend.append(lambda i=i: phase3b(i, on_act))
                if len(pend) > 1:
                    out.append(pend.pop(0))
            if defer_tail:
                return out, pend
            out.extend(pend)
            return out

        def interleave(a_items, b_items):
            out = []
            na, nb = len(a_items), len(b_items)
            bi = 0
            for ai, item in enumerate(a_items):
                out.append(item)
                want = int(round((ai + 1) * nb / na))
                while bi < want:
                    out.append(b_items[bi])
                    bi += 1
            out.extend(b_items[bi:])
            return out

        for it in proj_items(0) + proj_items(1):
            it()
        if carry:
            for it in carry[0]:
                it()
        attn_quarter(0)
        p2 = proj_items(2)
        vb8 = p2.pop(3)
        for it in p2:
            it()
        attn_quarter(1, inject={2: vb8})
        p3i = proj_items(3)
        vb12 = p3i.pop(3)
        for it in interleave(p3i, phase3_items(0, 4)):
            it()
        attn_quarter(2, inject={2: vb12})
        next_xT = None if last else xT_issue()
        items, tail = phase3_items(4, 8, defer_tail=True)
        for it in items:
            it()
        attn_quarter(3, prelude=tail + phase3_items(8, 12))
        deferred = phase3_items(12, 16, on_act=not last)
        if last:
            for it in deferred:
                it()
            return None
        return (deferred, next_xT)


# revision 7
# speedup vs baseline: 1.1395x; 1.1395x over previous
"""Multi-head causal attention (B=2, T=2048, E=768, H=12, D=64) on 8 trn2 cores.

Sharding: core c handles batch b=c//4 and heads [3g, 3g+1, 3g+2] (g=c%4).
Each core computes its 3 heads' attention plus their partial contribution to
the final projection; the host sums the 4 partials per batch.

v2 redesign (vs v1):
- all matmuls bf16 (1 cyc/row at any moving size; no fp32r small-N penalty)
- v computed directly in [s, d] layout (no PE transposes); v bias folded into
  phase 3 via a constant ones row in ot2e and a host-precomputed bv@wf row
- causal mask added via PE matmul (ident^T @ mask) inside the S^T psum
  accumulation group instead of a DVE tensor_tensor
- exp instructions batched over pairs of key blocks (2-bank PSUM st tiles)
- DMA issue on sync + gpsimd queues (keeps Act SEQ free for exp)
- phase-3 ob moves split DVE (cols 0:512) / Act (cols 512:768)
- fine-grained emission interleave (attention vs projection/phase-3 filler)
  to keep the PE stream dense

Per-core program:
  phase 1: qT/kT = Wqk^T x^T + b, column groups [q0 q1][q2 k0][k1 k2];
           v[s,d] = x W_v^T per 128-query block (3 heads side by side)
  phase 2: per head h, key-block j: S^T_j = k_j^T q (K=64), +mask on diagonal
           blocks via matmul, P = exp(scale*S^T) (Act, pair-batched),
           [O^T; l] accumulated via matmul(lhsT=[v_j | 1], rhs=P).
           recip = 1/l (DVE), partition-broadcast via K=1 matmul, multiply.
  phase 3: out = [ot01; ot2e]^T @ [wf01; wf2e] -> [2048, 768] partial, DMA.

`repeat` unrolls the whole body N times in one NEFF; test.py measures
per-body HW time as the slope of wall time vs repeat count.
"""
import numpy as np

EMBED_DIM = 768
B = 2
T = 2048
N_CORES = 8
NT = T // 128           # 16 query/key tiles
SCALE = 1.0 / np.sqrt(64.0)
NEG = -1.0e9


_state = {}


def _build(repeat=1):
    import concourse.tile as tile
    from concourse import bacc, mybir
    from concourse.masks import make_identity

    F32 = mybir.dt.float32
    BF16 = mybir.dt.bfloat16

    nc = bacc.Bacc("TRN2", target_bir_lowering=False, debug=False)

    xT_d = nc.dram_tensor("xT", [EMBED_DIM, T], BF16, kind="ExternalInput").ap()
    # columns ordered [q0 q1 | k0 k1 | q2 k2]
    wqk_d = nc.dram_tensor("wqk", [EMBED_DIM, 384], BF16, kind="ExternalInput").ap()
    wv_d = nc.dram_tensor("wv", [EMBED_DIM, 192], BF16, kind="ExternalInput").ap()
    bqk_d = nc.dram_tensor("bqk", [384, 1], F32, kind="ExternalInput").ap()
    wf01_d = nc.dram_tensor("wf01", [128, EMBED_DIM], BF16, kind="ExternalInput").ap()
    wf2e_d = nc.dram_tensor("wf2e", [65, EMBED_DIM], BF16, kind="ExternalInput").ap()
    mask_d = nc.dram_tensor("mask", [128, 128], BF16, kind="ExternalInput").ap()
    out_d = nc.dram_tensor("out_p", [T, EMBED_DIM], BF16, kind="ExternalOutput").ap()

    with tile.TileContext(nc) as tc:
        with tc.tile_pool(name="const", bufs=1) as const, \
             tc.tile_pool(name="persist", bufs=1) as persist:
            # ---- constants ----
            wqk_sb = const.tile([128, 6, 384], BF16)
            wv_sb = const.tile([128, 6, 192], BF16)
            nc.sync.dma_start(out=wqk_sb[:], in_=wqk_d.rearrange("(k p) c -> p k c", p=128))
            nc.gpsimd.dma_start(out=wv_sb[:], in_=wv_d.rearrange("(k p) c -> p k c", p=128))
            bqk_sb = [const.tile([128, 1], F32, name=f"bqk{m}", tag=f"bqk{m}")
                      for m in range(3)]
            for m in range(3):
                nc.sync.dma_start(out=bqk_sb[m][:], in_=bqk_d[128 * m:128 * (m + 1), :])
            wf01_sb = const.tile([128, EMBED_DIM], BF16)
            wf2e_sb = const.tile([65, EMBED_DIM], BF16)
            nc.gpsimd.dma_start(out=wf01_sb[:], in_=wf01_d[:])
            nc.gpsimd.dma_start(out=wf2e_sb[:], in_=wf2e_d[:])
            mask_sb = const.tile([128, 128], BF16)
            nc.sync.dma_start(out=mask_sb[:], in_=mask_d[:])
            ident_f = const.tile([128, 128], F32)
            make_identity(nc, ident_f)
            ident_bf = const.tile([128, 128], BF16)
            nc.vector.tensor_copy(out=ident_bf[:], in_=ident_f[:])
            ones_bf = const.tile([65, 64], BF16)
            nc.vector.memset(ones_bf[:], 1.0)

            # ---- persistent activations ----
            qA = persist.tile([128, T], BF16)    # q0 @0:64, q1 @64:128
            kA = persist.tile([128, T], BF16)    # k0 @0:64, k1 @64:128
            qB = persist.tile([64, T], BF16)     # q2
            kB = persist.tile([64, T], BF16)     # k2
            v_all = persist.tile([128, NT, 3, 65], BF16)   # [v | 1] per head
            nc.vector.memset(v_all[:, :, :, 64:65], 1.0)
            ot01 = persist.tile([128, T], BF16)  # normalized O^T h0 (@0), h1 (@64)
            ot2e = persist.tile([65, T], BF16)   # h2 @0:64; row 64 = ones
            nc.vector.memset(ot2e[64:65, :], 1.0)

            with tc.tile_pool(name="sb", bufs=1) as sbp, \
                 tc.tile_pool(name="ps", bufs=1, space="PSUM") as psp:
                carry = None
                for rep in range(repeat):
                    env = locals()
                    env.update(env.pop("env", {}) or {})
                    carry = _emit_body(nc, tc, rep, env, carry,
                                       last=(rep == repeat - 1))

    nc.compile()
    return nc


def _emit_body(nc, tc, rep, env, carry=None, last=True):
    """Emit one forward pass with fine-grained interleaving.

    `carry` holds the previous body's deferred phase-3 tail items; they are
    emitted after this body's first projection stage so the PE never idles
    waiting for the previous body's final softmax norms.  Returns this
    body's deferred items (or emits them when `last`).

    PSUM tags (8 banks): ps0 ps1 (projection/v/bc groups), stp0 stp1
    (2-bank S^T pair tiles), otl0 otl1 (PV accumulators + phase-3)."""
    from concourse import mybir

    F32 = mybir.dt.float32
    BF16 = mybir.dt.bfloat16
    Exp = mybir.ActivationFunctionType.Exp
    MULT = mybir.AluOpType.mult

    xT_d, out_d = env["xT_d"], env["out_d"]
    wqk_sb, wv_sb = env["wqk_sb"], env["wv_sb"]
    bqk_sb = env["bqk_sb"]
    wf01_sb, wf2e_sb = env["wf01_sb"], env["wf2e_sb"]
    ident_bf, ones_bf, mask_sb = env["ident_bf"], env["ones_bf"], env["mask_sb"]
    qA, kA, qB, kB = env["qA"], env["kA"], env["qB"], env["kB"]
    v_all = env["v_all"]
    ot01, ot2e = env["ot01"], env["ot2e"]
    dmae = [nc.sync, nc.gpsimd]
    sbp, psp = env["sbp"], env["psp"]

    if True:
        # ---- input DMA: xT as 6x4 chunks spread over 2 DMA queues ----
        # (first body issues its own; later bodies use tiles prefetched by
        # the previous body so the data is resident at body start)
        def xT_issue():
            tiles = [[sbp.tile([128, 512], BF16, name=f"xT{rep}_{k}_{n}",
                               tag=f"xT{k}{n}") for n in range(4)]
                     for k in range(6)]
            di = 0
            for n in range(4):
                for k in range(6):
                    dmae[di % 2].dma_start(
                        out=tiles[k][n][:],
                        in_=xT_d[128 * k:128 * (k + 1),
                                 512 * n:512 * (n + 1)])
                    di += 1
            return tiles

        if carry is not None and carry[1] is not None:
            xT_t = carry[1]
        else:
            xT_t = xT_issue()

        gidx = [0]

        def qk_group(m, n):
            ps = psp.tile([128, 512], F32, name=f"pg{rep}_{gidx[0]}",
                          tag=f"ps{gidx[0] % 2}")
            gidx[0] += 1
            for k in range(6):
                nc.tensor.matmul(ps[:], lhsT=wqk_sb[:, k, 128 * m:128 * (m + 1)],
                                 rhs=xT_t[k][n][:], start=(k == 0), stop=(k == 5))
            nsl = slice(512 * n, 512 * (n + 1))
            if m < 2:
                dst = qA if m == 0 else kA
                nc.vector.tensor_scalar_add(out=dst[:, nsl], in0=ps[:],
                                            scalar1=bqk_sb[m][:])
            else:
                nc.vector.tensor_scalar_add(out=qB[:, nsl], in0=ps[0:64, :],
                                            scalar1=bqk_sb[2][0:64, :])
                nc.vector.tensor_scalar_add(out=kB[:, nsl], in0=ps[64:128, :],
                                            scalar1=bqk_sb[2][64:128, :])

        def v_block(i):
            # v[s, d] for s-block i, 3 heads side by side: [128, 192]
            n, off = divmod(128 * i, 512)
            ps = psp.tile([128, 512], F32, name=f"vp{rep}_{i}",
                          tag=f"ps{gidx[0] % 2}")
            gidx[0] += 1
            for k in range(6):
                nc.tensor.matmul(ps[:, 0:192], lhsT=xT_t[k][n][:, off:off + 128],
                                 rhs=wv_sb[:, k, :], start=(k == 0), stop=(k == 5))
            nc.scalar.copy(
                out=v_all[:, i, :, 0:64],
                in_=ps[:, 0:192].rearrange("p (h d) -> p h d", h=3))

        # h -> (qT tile, q part offset, kT tile, k part offset)
        head_cfg = [(qA, 0, kA, 0), (qA, 64, kA, 64), (qB, 0, kB, 0)]
        sidx = [0]
        otli = [0]
        attn_st = {}

        def attn_S(h, q, inject=None):
            """S^T matmuls + diag mask + pair-batched exp for quarter q.
            inject: {pair_index: item} emitted after that pair, to absorb the
            exp pipeline phase lag without displacing the S stream."""
            qT, oq, kT, ok = head_cfg[h]
            base = 512 * q
            pairs = []
            for p in range(2 * q + 2):
                if inject and p in inject:
                    inject.pop(p)()
                st2 = psp.tile([128, 2, 512], F32, name=f"st{rep}_{h}{q}{p}",
                               tag=f"stp{sidx[0] % 2}")
                pt2 = sbp.tile([128, 2, 512], BF16, name=f"pt{rep}_{h}{q}{p}",
                               tag=f"pt{sidx[0] % 8}")
                sidx[0] += 1
                lns = []
                diags = []
                for jj in range(2):
                    j = 2 * p + jj
                    s0 = max(base, 128 * j)
                    ln = base + 512 - s0
                    lns.append(ln)
                    if 128 * j >= base:
                        diags.append(jj)
                    nc.tensor.matmul(
                        st2[:, jj, 0:ln],
                        lhsT=kT[ok:ok + 64, 128 * j:128 * (j + 1)],
                        rhs=qT[oq:oq + 64, s0:s0 + ln],
                        start=True, stop=True)
                mx = max(lns)
                nc.scalar.activation(out=pt2[:, :, 0:mx], in_=st2[:, :, 0:mx],
                                     func=Exp, scale=float(SCALE))
                for jj in diags:
                    nc.vector.tensor_tensor(
                        out=pt2[:, jj, 0:128], in0=pt2[:, jj, 0:128],
                        in1=mask_sb[:], op=MULT)
                pairs.append((st2, pt2, lns))
            attn_st[(h, q)] = pairs

        def attn_PV(h, q):
            base = 512 * q
            pairs = attn_st.pop((h, q))
            otl = psp.tile([128, 512], F32, name=f"otl{rep}_{h}{q}",
                           tag=f"otl{otli[0] % 2}")
            otli[0] += 1
            jmax = 4 * q + 3
            for j in range(jmax + 1):
                s0 = max(base, 128 * j)
                ln = base + 512 - s0
                pt2 = pairs[j // 2][1]
                nc.tensor.matmul(
                    otl[0:65, s0 - base:512],
                    lhsT=v_all[:, j, h, :], rhs=pt2[:, j % 2, 0:ln],
                    start=(j == 0), stop=(j == jmax))
            return otl

        def attn_norm(h, q, otl):
            base = 512 * q
            recip = sbp.tile([65, 512], F32, name=f"rc{rep}_{h}{q}",
                             tag=f"rc{(3 * q + h) % 2}")
            nc.vector.reciprocal(out=recip[0:1, :], in_=otl[64:65, :])
            bcs = sbp.tile([64, 512], F32, name=f"bcs{rep}_{h}{q}",
                           tag=f"bcs{(3 * q + h) % 2}")
            nc.gpsimd.partition_broadcast(bcs[:, :], recip[0:1, :])
            dst, od = (ot2e, 0) if h == 2 else (ot01, 64 * h)
            nc.vector.tensor_tensor(
                out=dst[od:od + 64, base:base + 512],
                in0=otl[0:64, :], in1=bcs[:], op=MULT)

        p3_ob = {}

        def phase3a(i, on_act=False):
            fpa = psp.tile([128, 512], F32, name=f"fpa{rep}_{i}", tag=f"otl{i % 2}")
            ti = slice(128 * i, 128 * (i + 1))
            nc.tensor.matmul(fpa[:], lhsT=ot01[:, ti],
                             rhs=wf01_sb[:, 0:512], start=True, stop=False)
            nc.tensor.matmul(fpa[:], lhsT=ot2e[:, ti],
                             rhs=wf2e_sb[:, 0:512], start=False, stop=True)
            ob = sbp.tile([128, EMBED_DIM], BF16, name=f"ob{rep}_{i}",
                          tag=f"ob{i % 6}")
            p3_ob[i] = ob
            if on_act:
                nc.scalar.copy(out=ob[:, 0:512], in_=fpa[:, :])
            else:
                nc.vector.tensor_copy(out=ob[:, 0:512], in_=fpa[:, :])

        def phase3b(i, on_act=False):
            fpb = psp.tile([128, 512], F32, name=f"fpb{rep}_{i}", tag=f"otl{i % 2}")
            ti = slice(128 * i, 128 * (i + 1))
            nc.tensor.matmul(fpb[:, 0:256], lhsT=ot01[:, ti],
                             rhs=wf01_sb[:, 512:768], start=True, stop=False)
            nc.tensor.matmul(fpb[:, 0:256], lhsT=ot2e[:, ti],
                             rhs=wf2e_sb[:, 512:768], start=False, stop=True)
            ob = p3_ob.pop(i)
            if on_act:
                nc.scalar.copy(out=ob[:, 512:768], in_=fpb[:, 0:256])
            else:
                nc.vector.tensor_copy(out=ob[:, 512:768], in_=fpb[:, 0:256])
            dmae[i % 2].dma_start(out=out_d[ti, :], in_=ob[:])

        # ---- staged emission ----
        # Safety rule: attention quarters are emitted as contiguous blocks
        # (their st/pt/otl tag rotations + cross-engine deps form cycles if
        # other otl-tag users are woven in).  Projections and phase-3 use
        # disjoint tag sets, so they can interleave with each other freely.
        def attn_quarter(q, prelude=(), inject=None):
            """Attention for quarter q, inter-head pipelined.  `prelude` items
            (phase-3 leftovers) are emitted after the first S batch so their
            psum-tag waits hide behind this quarter's exp latency."""
            otl = {}
            attn_S(0, q, inject)
            for it in prelude:
                it()
            attn_S(1, q)
            otl[0] = attn_PV(0, q)
            attn_S(2, q)
            otl[1] = attn_PV(1, q)
            attn_norm(0, q, otl.pop(0))
            otl[2] = attn_PV(2, q)
            attn_norm(1, q, otl.pop(1))
            attn_norm(2, q, otl.pop(2))

        def proj_items(n):
            items = [lambda m=m: qk_group(m, n) for m in range(3)]
            items += [lambda i=i: v_block(i) for i in range(4 * n, 4 * n + 4)]
            return items

        def phase3_items(i0, i1, defer_tail=False, on_act=False):
            """a/b sub-items pipelined: a(i), a(i+1), b(i), a(i+2), b(i+1)...
            With defer_tail, the trailing b-items are split off for the
            caller to emit later."""
            out = []
            pend = []
            for i in range(i0, i1):
                out.append(lambda i=i: phase3a(i, on_act))
                pend.append(lambda i=i: phase3b(i, on_act))
                if len(pend) > 1:
                    out.append(pend.pop(0))
            if defer_tail:
                return out, pend
            out.extend(pend)
            return out

        def interleave(a_items, b_items):
            out = []
            na, nb = len(a_items), len(b_items)
            bi = 0
            for ai, item in enumerate(a_items):
                out.append(item)
                want = int(round((ai + 1) * nb / na))
                while bi < want:
                    out.append(b_items[bi])
                    bi += 1
            out.extend(b_items[bi:])
            return out

        for it in proj_items(0) + proj_items(1):
            it()
        if carry:
            for it in carry[0]:
                it()
        attn_quarter(0)
        p2 = proj_items(2)
        vb8 = p2.pop(3)
        for it in p2:
            it()
        attn_quarter(1, inject={2: vb8})
        p3i = proj_items(3)
        vb12 = p3i.pop(3)
        for it in interleave(p3i, phase3_items(0, 4)):
            it()
        attn_quarter(2, inject={2: vb12})
        next_xT = None if last else xT_issue()
        items, tail = phase3_items(4, 8, defer_tail=True)
        for it in items:
            it()
        attn_quarter(3, prelude=tail + phase3_items(8, 12))
        deferred = phase3_items(12, 16, on_act=not last)
        if last:
            for it in deferred:
                it()
            return None
        return (deferred, next_xT)
